# revision 10
# baseline (speedup 1.0000x reference)
"""Trainium2 Bass kernel for the AdditiveModel reduction.

Computes out[y] = sum_{q,p} c[y,q] * a[y,q,p] * dot(lam[y,q,p,:], x[q,p,:])
with Y=16, Q=8, P=32, D=8192 (lam is 128 MiB -> memory-bound).

Sharding: one q per core (Q == 8 cores). Each core is fully independent and
produces a partial out[16]; the host sums the 8 partials at gather time.

Per-core compute: the D-axis dot products run on the TensorEngine. At
sharding time the host hands each core its lam slice pre-transposed to
[d, (y,p)] (d on partitions), so the dots become 64 PSUM-accumulated
matmuls lhsT=x[dchunk, p] (128x32), rhs=lam[dchunk, (y,p)] (128x512).
PSUM then holds G[m, (y,p)] = dot(x[p_m,:], lam[y,p,:]); a masked
diagonal extraction + (c*a) weighting + ones-matmul collapse yields out.
"""

from contextlib import ExitStack

import numpy as np

Y, Q, P, D = 16, 8, 32, 8192
NCORES = 8
KC = 128                 # contraction chunk (partition count)
DC = D // KC             # 64 d-chunks
SLABS = 8                # lam streamed in 8 slabs of 2 MiB
CPS = DC // SLABS        # chunks per slab
YP = Y * P               # 512

_CACHE = {}


def _build_nc():
    import concourse.bass as bass
    import concourse.mybir as mybir
    import concourse.tile as tile
    from concourse import bacc

    f32 = mybir.dt.float32
    nc = bacc.Bacc(None, target_bir_lowering=False)

    f32r = mybir.dt.float32r
    lamT = nc.declare_dram_parameter("lamT", [KC, DC * YP], f32r, isOutput=False)
    xT = nc.declare_dram_parameter("xT", [KC, DC * P], f32r, isOutput=False)
    aT = nc.declare_dram_parameter("aT", [P, Y], f32, isOutput=False)
    crep = nc.declare_dram_parameter("crep", [P, Y], f32, isOutput=False)
    m0 = nc.declare_dram_parameter("m0", [P, YP], f32, isOutput=False)
    ones = nc.declare_dram_parameter("ones", [P, 1], f32, isOutput=False)
    out = nc.declare_dram_parameter("out", [1, Y], f32, isOutput=True)

    with tile.TileContext(nc) as tc, ExitStack() as ctx:
        const = ctx.enter_context(tc.tile_pool(name="const", bufs=1))
        slab_pool = ctx.enter_context(tc.tile_pool(name="slab", bufs=3))
        psum_pool = ctx.enter_context(
            tc.tile_pool(name="psum", bufs=1, space=bass.MemorySpace.PSUM)
        )
        tailp = ctx.enter_context(tc.tile_pool(name="tail", bufs=1))

        # lam slabs stream on the SP HWDGE ring; everything else loads via
        # the ACT ring so the first slab's descriptors hit DMA immediately.
        x_sb = const.tile([KC, DC * P], f32r)
        nc.scalar.dma_start(x_sb[:], xT[:])
        m0_sb = const.tile([P, YP], f32)
        nc.scalar.dma_start(m0_sb[:], m0[:])
        aT_sb = const.tile([P, Y], f32)
        nc.scalar.dma_start(aT_sb[:], aT[:])
        cr_sb = const.tile([P, Y], f32)
        nc.scalar.dma_start(cr_sb[:], crep[:])
        on_sb = const.tile([P, 1], f32)
        nc.scalar.dma_start(on_sb[:], ones[:])

        wT = const.tile([P, Y], f32)
        nc.vector.tensor_mul(wT[:], aT_sb[:], cr_sb[:])

        proj = psum_pool.tile([P, YP], f32)
        for s in range(SLABS):
            slab = slab_pool.tile([KC, CPS * YP], f32r)
            eng = nc.sync if s % 2 == 0 else nc.scalar
            eng.dma_start(slab[:], lamT[:, s * CPS * YP:(s + 1) * CPS * YP])
            for c in range(CPS):
                cg = s * CPS + c
                nc.tensor.matmul(
                    proj[:],
                    x_sb[:, cg * P:(cg + 1) * P],
                    slab[:, c * YP:(c + 1) * YP],
                    start=(cg == 0),
                    stop=(cg == DC - 1),
                )

        # diag mask: keep only m == p entries of G[m, (y,p)]
        t2 = tailp.tile([P, YP], f32)
        nc.vector.tensor_mul(t2[:], proj[:], m0_sb[:])
        # sum each 32-wide p-group -> S[m, y] = proj[y, m]
        s_t = tailp.tile([P, Y], f32)
        nc.vector.reduce_sum(
            s_t[:],
            t2[:].rearrange("m (y p) -> m y p", p=P),
            axis=mybir.AxisListType.X,
        )
        # weight by c*a and collapse partitions with a ones-matvec
        sw = tailp.tile([P, Y], f32)
        nc.vector.tensor_mul(sw[:], s_t[:], wT[:])
        outp = psum_pool.tile([1, Y], f32)
        nc.tensor.matmul(outp[:], on_sb[:], sw[:], start=True, stop=True)
        out_sb = tailp.tile([1, Y], f32)
        nc.vector.tensor_copy(out_sb[:], outp[:])
        nc.sync.dma_start(out[:], out_sb[:])

    nc.compile()
    return nc


def _shard_inputs(x, lam, a, c):
    """Per-core input maps. Pure slicing/layout transforms only."""
    m0_np = np.tile(np.eye(P, dtype=np.float32), (1, Y))          # [P, Y*P]
    ones_np = np.ones((P, 1), dtype=np.float32)
    in_maps = []
    for q in range(NCORES):
        lam_q = lam[:, q]                                          # [Y, P, D]
        lamT = np.ascontiguousarray(
            lam_q.transpose(2, 0, 1).reshape(DC, KC, YP)
            .transpose(1, 0, 2).reshape(KC, DC * YP)
        )
        x_q = x[q]                                                 # [P, D]
        xTn = np.ascontiguousarray(
            x_q.T.reshape(DC, KC, P).transpose(1, 0, 2).reshape(KC, DC * P)
        )
        aTn = np.ascontiguousarray(a[:, q].T)                      # [P, Y]
        crn = np.ascontiguousarray(
            np.broadcast_to(c[:, q][None, :], (P, Y))
        ).astype(np.float32)
        in_maps.append(
            {
                "lamT": lamT.astype(np.float32, copy=False),
                "xT": xTn.astype(np.float32, copy=False),
                "aT": aTn.astype(np.float32, copy=False),
                "crep": crn,
                "m0": m0_np,
                "ones": ones_np,
            }
        )
    return in_maps


def get_nc():
    if "nc" not in _CACHE:
        _CACHE["nc"] = _build_nc()
    return _CACHE["nc"]


def run(x, lam, a, c, trace=False, **spmd_kwargs):
    from concourse.bass_utils import run_bass_kernel_spmd

    nc = get_nc()
    in_maps = _shard_inputs(
        np.asarray(x), np.asarray(lam), np.asarray(a), np.asarray(c)
    )
    res = run_bass_kernel_spmd(
        nc, in_maps, core_ids=list(range(NCORES)), trace=trace, **spmd_kwargs
    )
    out = np.zeros((Y,), dtype=np.float32)
    for core_res in res.results:
        out += core_res["out"].reshape(Y)
    return out, res


def kernel(x, lam, a, c):
    out, _ = run(x, lam, a, c, trace=False)
    return out


# revision 11
# speedup vs baseline: 1.2125x; 1.2125x over previous
"""Trainium2 Bass kernel for the AdditiveModel reduction.

Computes out[y] = sum_{q,p} c[y,q] * a[y,q,p] * dot(lam[y,q,p,:], x[q,p,:])
with Y=16, Q=8, P=32, D=8192 (lam is 128 MiB -> memory-bound).

Sharding: one q per core (Q == 8 cores). Each core is fully independent and
produces a partial out[16]; the host sums the 8 partials at gather time.

Per-core compute: the D-axis dot products run on the TensorEngine. At
sharding time the host hands each core its lam slice pre-transposed to
[d, (y,p)] (d on partitions), so the dots become 64 PSUM-accumulated
matmuls lhsT=x[dchunk, p] (128x32), rhs=lam[dchunk, (y,p)] (128x512).
PSUM then holds G[m, (y,p)] = dot(x[p_m,:], lam[y,p,:]); a masked
diagonal extraction + (c*a) weighting + ones-matmul collapse yields out.
"""

from contextlib import ExitStack

import numpy as np

Y, Q, P, D = 16, 8, 32, 8192
NCORES = 8
KC = 128                 # contraction chunk (partition count)
DC = D // KC             # 64 d-chunks
SLABS = 8                # lam streamed in 8 slabs of 2 MiB
CPS = DC // SLABS        # chunks per slab
YP = Y * P               # 512

_CACHE = {}


def _build_nc():
    import concourse.bass as bass
    import concourse.mybir as mybir
    import concourse.tile as tile
    from concourse import bacc

    f32 = mybir.dt.float32
    nc = bacc.Bacc(None, target_bir_lowering=False)

    f32r = mybir.dt.float32r
    lamT = nc.declare_dram_parameter("lamT", [KC, DC * YP], f32r, isOutput=False)
    xT = nc.declare_dram_parameter("xT", [KC, DC * P], f32r, isOutput=False)
    aT = nc.declare_dram_parameter("aT", [P, Y], f32, isOutput=False)
    crep = nc.declare_dram_parameter("crep", [P, Y], f32, isOutput=False)
    m0 = nc.declare_dram_parameter("m0", [P, YP], f32, isOutput=False)
    ones = nc.declare_dram_parameter("ones", [P, 1], f32, isOutput=False)
    out = nc.declare_dram_parameter("out", [1, Y], f32, isOutput=True)

    with tile.TileContext(nc) as tc, ExitStack() as ctx:
        const = ctx.enter_context(tc.tile_pool(name="const", bufs=1))
        slab_pool = ctx.enter_context(tc.tile_pool(name="slab", bufs=3))
        psum_pool = ctx.enter_context(
            tc.tile_pool(name="psum", bufs=1, space=bass.MemorySpace.PSUM)
        )
        tailp = ctx.enter_context(tc.tile_pool(name="tail", bufs=1))

        # lam slabs stream on the SP HWDGE ring; everything else loads via
        # the ACT ring so the first slab's descriptors hit DMA immediately.
        x_sb = const.tile([KC, DC * P], f32r)
        nc.scalar.dma_start(x_sb[:], xT[:])
        m0_sb = const.tile([P, YP], f32)
        nc.scalar.dma_start(m0_sb[:], m0[:])
        aT_sb = const.tile([P, Y], f32)
        nc.scalar.dma_start(aT_sb[:], aT[:])
        cr_sb = const.tile([P, Y], f32)
        nc.scalar.dma_start(cr_sb[:], crep[:])
        on_sb = const.tile([P, 1], f32)
        nc.scalar.dma_start(on_sb[:], ones[:])

        wT = const.tile([P, Y], f32)
        nc.vector.tensor_mul(wT[:], aT_sb[:], cr_sb[:])

        proj = psum_pool.tile([P, YP], f32)
        for s in range(SLABS):
            slab = slab_pool.tile([KC, CPS * YP], f32r)
            nc.sync.dma_start(slab[:], lamT[:, s * CPS * YP:(s + 1) * CPS * YP])
            for c in range(CPS):
                cg = s * CPS + c
                nc.tensor.matmul(
                    proj[:],
                    x_sb[:, cg * P:(cg + 1) * P],
                    slab[:, c * YP:(c + 1) * YP],
                    start=(cg == 0),
                    stop=(cg == DC - 1),
                )

        # diag mask: keep only m == p entries of G[m, (y,p)]
        t2 = tailp.tile([P, YP], f32)
        nc.vector.tensor_mul(t2[:], proj[:], m0_sb[:])
        # sum each 32-wide p-group -> S[m, y] = proj[y, m]
        s_t = tailp.tile([P, Y], f32)
        nc.vector.reduce_sum(
            s_t[:],
            t2[:].rearrange("m (y p) -> m y p", p=P),
            axis=mybir.AxisListType.X,
        )
        # weight by c*a and collapse partitions with a ones-matvec
        sw = tailp.tile([P, Y], f32)
        nc.vector.tensor_mul(sw[:], s_t[:], wT[:])
        outp = psum_pool.tile([1, Y], f32)
        nc.tensor.matmul(outp[:], on_sb[:], sw[:], start=True, stop=True)
        out_sb = tailp.tile([1, Y], f32)
        nc.vector.tensor_copy(out_sb[:], outp[:])
        nc.sync.dma_start(out[:], out_sb[:])

    nc.compile()
    return nc


def _shard_inputs(x, lam, a, c):
    """Per-core input maps. Pure slicing/layout transforms only."""
    m0_np = np.tile(np.eye(P, dtype=np.float32), (1, Y))          # [P, Y*P]
    ones_np = np.ones((P, 1), dtype=np.float32)
    in_maps = []
    for q in range(NCORES):
        lam_q = lam[:, q]                                          # [Y, P, D]
        lamT = np.ascontiguousarray(
            lam_q.transpose(2, 0, 1).reshape(DC, KC, YP)
            .transpose(1, 0, 2).reshape(KC, DC * YP)
        )
        x_q = x[q]                                                 # [P, D]
        xTn = np.ascontiguousarray(
            x_q.T.reshape(DC, KC, P).transpose(1, 0, 2).reshape(KC, DC * P)
        )
        aTn = np.ascontiguousarray(a[:, q].T)                      # [P, Y]
        crn = np.ascontiguousarray(
            np.broadcast_to(c[:, q][None, :], (P, Y))
        ).astype(np.float32)
        in_maps.append(
            {
                "lamT": lamT.astype(np.float32, copy=False),
                "xT": xTn.astype(np.float32, copy=False),
                "aT": aTn.astype(np.float32, copy=False),
                "crep": crn,
                "m0": m0_np,
                "ones": ones_np,
            }
        )
    return in_maps


def get_nc():
    if "nc" not in _CACHE:
        _CACHE["nc"] = _build_nc()
    return _CACHE["nc"]


def run(x, lam, a, c, trace=False, **spmd_kwargs):
    from concourse.bass_utils import run_bass_kernel_spmd

    nc = get_nc()
    in_maps = _shard_inputs(
        np.asarray(x), np.asarray(lam), np.asarray(a), np.asarray(c)
    )
    res = run_bass_kernel_spmd(
        nc, in_maps, core_ids=list(range(NCORES)), trace=trace, **spmd_kwargs
    )
    out = np.zeros((Y,), dtype=np.float32)
    for core_res in res.results:
        out += core_res["out"].reshape(Y)
    return out, res


def kernel(x, lam, a, c):
    out, _ = run(x, lam, a, c, trace=False)
    return out


# revision 12
# speedup vs baseline: 1.5601x; 1.2868x over previous
"""Trainium2 Bass kernel for the AdditiveModel reduction.

Computes out[y] = sum_{q,p} c[y,q] * a[y,q,p] * dot(lam[y,q,p,:], x[q,p,:])
with Y=16, Q=8, P=32, D=8192 (lam is 128 MiB -> memory-bound).

Sharding: one q per core (Q == 8 cores). Each core is fully independent and
produces a partial out[16]; the host sums the 8 partials at gather time.

Per-core compute: the D-axis dot products run on the TensorEngine. At
sharding time the host hands each core its lam slice pre-transposed to
[d, (y,p)] (d on partitions), so the dots become 64 PSUM-accumulated
matmuls lhsT=x[dchunk, p] (128x32), rhs=lam[dchunk, (y,p)] (128x512).
PSUM then holds G[m, (y,p)] = dot(x[p_m,:], lam[y,p,:]); a masked
diagonal extraction + (c*a) weighting + ones-matmul collapse yields out.
"""

from contextlib import ExitStack

import numpy as np

Y, Q, P, D = 16, 8, 32, 8192
NCORES = 8
KC = 128                 # contraction chunk (partition count)
DC = D // KC             # 64 d-chunks
SLABS = 8                # lam streamed in 8 slabs of 2 MiB
CPS = DC // SLABS        # chunks per slab
YP = Y * P               # 512

_CACHE = {}


def _build_nc():
    import concourse.bass as bass
    import concourse.mybir as mybir
    import concourse.tile as tile
    from concourse import bacc

    f32 = mybir.dt.float32
    nc = bacc.Bacc(None, target_bir_lowering=False)

    f16 = mybir.dt.float16
    lamT = nc.declare_dram_parameter("lamT", [KC, DC * YP], f16, isOutput=False)
    xT = nc.declare_dram_parameter("xT", [KC, DC * P], f16, isOutput=False)
    aT = nc.declare_dram_parameter("aT", [P, Y], f32, isOutput=False)
    crep = nc.declare_dram_parameter("crep", [P, Y], f32, isOutput=False)
    m0 = nc.declare_dram_parameter("m0", [P, YP], f32, isOutput=False)
    ones = nc.declare_dram_parameter("ones", [P, 1], f32, isOutput=False)
    out = nc.declare_dram_parameter("out", [1, Y], f32, isOutput=True)

    with tile.TileContext(nc) as tc, ExitStack() as ctx:
        const = ctx.enter_context(tc.tile_pool(name="const", bufs=1))
        slab_pool = ctx.enter_context(tc.tile_pool(name="slab", bufs=3))
        psum_pool = ctx.enter_context(
            tc.tile_pool(name="psum", bufs=1, space=bass.MemorySpace.PSUM)
        )
        tailp = ctx.enter_context(tc.tile_pool(name="tail", bufs=1))

        # lam slabs stream on the SP HWDGE ring; everything else loads via
        # the ACT ring so the first slab's descriptors hit DMA immediately.
        x_sb = const.tile([KC, DC * P], f16)
        nc.scalar.dma_start(x_sb[:], xT[:])
        m0_sb = const.tile([P, YP], f32)
        nc.scalar.dma_start(m0_sb[:], m0[:])
        aT_sb = const.tile([P, Y], f32)
        nc.scalar.dma_start(aT_sb[:], aT[:])
        cr_sb = const.tile([P, Y], f32)
        nc.scalar.dma_start(cr_sb[:], crep[:])
        on_sb = const.tile([P, 1], f32)
        nc.scalar.dma_start(on_sb[:], ones[:])

        wT = const.tile([P, Y], f32)
        nc.vector.tensor_mul(wT[:], aT_sb[:], cr_sb[:])

        proj = psum_pool.tile([P, YP], f32)
        for s in range(SLABS):
            slab = slab_pool.tile([KC, CPS * YP], f16)
            nc.sync.dma_start(slab[:], lamT[:, s * CPS * YP:(s + 1) * CPS * YP])
            for c in range(CPS):
                cg = s * CPS + c
                nc.tensor.matmul(
                    proj[:],
                    x_sb[:, cg * P:(cg + 1) * P],
                    slab[:, c * YP:(c + 1) * YP],
                    start=(cg == 0),
                    stop=(cg == DC - 1),
                )

        # diag mask: keep only m == p entries of G[m, (y,p)]
        t2 = tailp.tile([P, YP], f32)
        nc.vector.tensor_mul(t2[:], proj[:], m0_sb[:])
        # sum each 32-wide p-group -> S[m, y] = proj[y, m]
        s_t = tailp.tile([P, Y], f32)
        nc.vector.reduce_sum(
            s_t[:],
            t2[:].rearrange("m (y p) -> m y p", p=P),
            axis=mybir.AxisListType.X,
        )
        # weight by c*a and collapse partitions with a ones-matvec
        sw = tailp.tile([P, Y], f32)
        nc.vector.tensor_mul(sw[:], s_t[:], wT[:])
        outp = psum_pool.tile([1, Y], f32)
        nc.tensor.matmul(outp[:], on_sb[:], sw[:], start=True, stop=True)
        out_sb = tailp.tile([1, Y], f32)
        nc.vector.tensor_copy(out_sb[:], outp[:])
        nc.sync.dma_start(out[:], out_sb[:])

    nc.compile()
    return nc


def _shard_inputs(x, lam, a, c):
    """Per-core input maps. Pure slicing/layout transforms only."""
    m0_np = np.tile(np.eye(P, dtype=np.float32), (1, Y))          # [P, Y*P]
    ones_np = np.ones((P, 1), dtype=np.float32)
    in_maps = []
    for q in range(NCORES):
        lam_q = lam[:, q]                                          # [Y, P, D]
        lamT = np.ascontiguousarray(
            lam_q.transpose(2, 0, 1).reshape(DC, KC, YP)
            .transpose(1, 0, 2).reshape(KC, DC * YP)
        )
        x_q = x[q]                                                 # [P, D]
        xTn = np.ascontiguousarray(
            x_q.T.reshape(DC, KC, P).transpose(1, 0, 2).reshape(KC, DC * P)
        )
        aTn = np.ascontiguousarray(a[:, q].T)                      # [P, Y]
        crn = np.ascontiguousarray(
            np.broadcast_to(c[:, q][None, :], (P, Y))
        ).astype(np.float32)
        in_maps.append(
            {
                "lamT": lamT.astype(np.float16),
                "xT": xTn.astype(np.float16),
                "aT": aTn.astype(np.float32, copy=False),
                "crep": crn,
                "m0": m0_np,
                "ones": ones_np,
            }
        )
    return in_maps


def get_nc():
    if "nc" not in _CACHE:
        _CACHE["nc"] = _build_nc()
    return _CACHE["nc"]


def run(x, lam, a, c, trace=False, **spmd_kwargs):
    from concourse.bass_utils import run_bass_kernel_spmd

    nc = get_nc()
    in_maps = _shard_inputs(
        np.asarray(x), np.asarray(lam), np.asarray(a), np.asarray(c)
    )
    res = run_bass_kernel_spmd(
        nc, in_maps, core_ids=list(range(NCORES)), trace=trace, **spmd_kwargs
    )
    out = np.zeros((Y,), dtype=np.float32)
    for core_res in res.results:
        out += core_res["out"].reshape(Y)
    return out, res


def kernel(x, lam, a, c):
    out, _ = run(x, lam, a, c, trace=False)
    return out


# revision 21
# speedup vs baseline: 1.6506x; 1.0580x over previous
"""Trainium2 Bass kernel for the AdditiveModel reduction.

Computes out[y] = sum_{q,p} c[y,q] * a[y,q,p] * dot(lam[y,q,p,:], x[q,p,:])
with Y=16, Q=8, P=32, D=8192 (lam is 128 MiB -> memory-bound).

Sharding: one q per core (Q == 8 cores). Each core is fully independent and
produces a partial out[16]; the host sums the 8 partials at gather time.

Per-core compute: the D-axis dot products run on the TensorEngine. At
sharding time the host hands each core its lam slice pre-transposed to
[d, (y,p)] layout (d on partitions) and cast to fp16 (error ~1e-4 of output
scale, matching the fp32r PE path), so the dots become 64 PSUM-accumulated
matmuls lhsT=x[dchunk, p] (128x32), rhs=lam[dchunk, (y,p)] (128x512) with
fp32 accumulation. PSUM holds G[m, (y,p)] = dot(x[p_m,:], lam[y,p,:]); a
masked diagonal extraction + (c*a) weighting + ones-matvec collapse yields
the 16 outputs. The accumulation is split into a main group and a small
final group so most of the tail reduction hides behind the last chunks'
DMA.
"""

from contextlib import ExitStack

import numpy as np

Y, Q, P, D = 16, 8, 32, 8192
NCORES = 8
KC = 128                 # contraction chunk (partition count)
DC = D // KC             # 64 d-chunks
YP = Y * P               # 512

# streaming config (tuned on HW)
MODE = "dual"            # "single": all lam on SP ring; "dual": split SP+ACT
SLABS = 8                # lam DMA count (per ring in dual mode)
BUFS = None              # slab pool slots; None -> all slabs resident
TAIL_PAIRS = 4           # last chunk-pairs accumulated in a second psum tile
XSPLIT = 4               # x loaded in this many piecewise DMAs

_CACHE = {}


def _build_nc():
    import concourse.bass as bass
    import concourse.mybir as mybir
    import concourse.tile as tile
    from concourse import bacc

    f32 = mybir.dt.float32
    f16 = mybir.dt.float16
    nc = bacc.Bacc(None, target_bir_lowering=False)

    lamT = nc.declare_dram_parameter("lamT", [KC, DC * YP], f16, isOutput=False)
    xT = nc.declare_dram_parameter("xT", [KC, DC * P], f16, isOutput=False)
    aT = nc.declare_dram_parameter("aT", [P, Y], f32, isOutput=False)
    crep = nc.declare_dram_parameter("crep", [P, Y], f32, isOutput=False)
    m0 = nc.declare_dram_parameter("m0", [P, YP], f32, isOutput=False)
    ones = nc.declare_dram_parameter("ones", [P, 1], f32, isOutput=False)
    out = nc.declare_dram_parameter("out", [1, Y], f32, isOutput=True)

    bufs = BUFS if BUFS is not None else SLABS * (2 if MODE == "dual" else 1)

    with tile.TileContext(nc) as tc, ExitStack() as ctx:
        const = ctx.enter_context(tc.tile_pool(name="const", bufs=1))
        slab_pool = ctx.enter_context(tc.tile_pool(name="slab", bufs=bufs))
        psum_pool = ctx.enter_context(
            tc.tile_pool(name="psum", bufs=1, space=bass.MemorySpace.PSUM)
        )
        tailp = ctx.enter_context(tc.tile_pool(name="tail", bufs=1))

        # x loads piecewise on the SP HWDGE ring so the first matmul only
        # waits for its own slice. SWDGE (gpsimd) is avoided everywhere: its
        # Q7 descriptor generation starts ~5us late.
        x_sb = const.tile([KC, DC * P], f16)
        xcp = DC // XSPLIT * P
        nc.sync.dma_start(x_sb[:, 0:P], xT[:, 0:P])
        for i in range(XSPLIT):
            lo = i * xcp + (P if i == 0 else 0)
            nc.sync.dma_start(x_sb[:, lo:(i + 1) * xcp], xT[:, lo:(i + 1) * xcp])

        proj = psum_pool.tile([P, YP], f32)    # main accumulation group
        proj2 = psum_pool.tile([P, YP], f32)   # last TAIL_PAIRS group

        def emit_mm(cg, slab_ap, first, last):
            nc.tensor.matmul(
                proj2[:] if last else proj[:],
                x_sb[:, cg * P:(cg + 1) * P],
                slab_ap,
                start=first,
                stop=last and cg == _last_cg,
            )

        if MODE == "single":
            cps = DC // SLABS
            order = [(s, c) for s in range(SLABS) for c in range(cps)]
            cut = DC - 2 * TAIL_PAIRS
            _last_cg = DC - 1
            slabs = {}
            first_main, first_tail = True, True
            for s in range(SLABS):
                slab = slab_pool.tile([KC, cps * YP], f16)
                nc.sync.dma_start(
                    slab[:], lamT[:, s * cps * YP:(s + 1) * cps * YP]
                )
                slabs[s] = slab
            # interleaving handled below
            mm_seq = []
            for s in range(SLABS):
                for c in range(cps):
                    cg = s * cps + c
                    mm_seq.append((cg, slabs[s][:, c * YP:(c + 1) * YP]))
        else:
            half = DC // 2
            cps = half // SLABS
            cut = DC - 2 * TAIL_PAIRS
            _last_cg = DC - 1
            mm_seq = []
            for s in range(SLABS):
                slab_a = slab_pool.tile([KC, cps * YP], f16, tag="slab_a")
                a_lo = s * cps
                nc.sync.dma_start(slab_a[:], lamT[:, a_lo * YP:(a_lo + cps) * YP])
                slab_b = slab_pool.tile([KC, cps * YP], f16, tag="slab_b")
                b_lo = half + s * cps
                nc.scalar.dma_start(
                    slab_b[:], lamT[:, b_lo * YP:(b_lo + cps) * YP]
                )
                for c in range(cps):
                    mm_seq.append((a_lo + c, slab_a[:, c * YP:(c + 1) * YP]))
                    mm_seq.append((b_lo + c, slab_b[:, c * YP:(c + 1) * YP]))

        # emit matmuls in stream order; the last 2*TAIL_PAIRS go to proj2
        n_tail = 2 * TAIL_PAIRS
        n_main = len(mm_seq) - n_tail
        for i, (cg, ap) in enumerate(mm_seq):
            to_tail = i >= n_main
            dst = proj2 if to_tail else proj
            nc.tensor.matmul(
                dst[:],
                x_sb[:, cg * P:(cg + 1) * P],
                ap,
                start=(i == 0) or (to_tail and i == n_main),
                stop=(i == n_main - 1) or (i == len(mm_seq) - 1),
            )

        # consts ride the ACT ring (needed only for the tail)
        m0_sb = const.tile([P, YP], f32)
        nc.scalar.dma_start(m0_sb[:], m0[:])
        aT_sb = const.tile([P, Y], f32)
        nc.scalar.dma_start(aT_sb[:], aT[:])
        cr_sb = const.tile([P, Y], f32)
        nc.scalar.dma_start(cr_sb[:], crep[:])
        on_sb = const.tile([P, 1], f32)
        nc.scalar.dma_start(on_sb[:], ones[:])
        wT = const.tile([P, Y], f32)
        nc.vector.tensor_mul(wT[:], aT_sb[:], cr_sb[:])

        # tail part 1 (main group): overlaps the last chunks' DMA/matmuls
        t2 = tailp.tile([P, YP], f32)
        nc.vector.tensor_mul(t2[:], proj[:], m0_sb[:])
        s_t = tailp.tile([P, Y], f32)
        nc.vector.reduce_sum(
            s_t[:],
            t2[:].rearrange("m (y p) -> m y p", p=P),
            axis=mybir.AxisListType.X,
        )
        # tail part 2 (last chunks)
        t2b = tailp.tile([P, YP], f32)
        nc.vector.tensor_mul(t2b[:], proj2[:], m0_sb[:])
        s_tb = tailp.tile([P, Y], f32)
        nc.vector.reduce_sum(
            s_tb[:],
            t2b[:].rearrange("m (y p) -> m y p", p=P),
            axis=mybir.AxisListType.X,
        )
        s_sum = tailp.tile([P, Y], f32)
        nc.vector.tensor_add(s_sum[:], s_t[:], s_tb[:])
        # weight by c*a and collapse partitions with a ones-matvec
        sw = tailp.tile([P, Y], f32)
        nc.vector.tensor_mul(sw[:], s_sum[:], wT[:])
        outp = psum_pool.tile([1, Y], f32)
        nc.tensor.matmul(outp[:], on_sb[:], sw[:], start=True, stop=True)
        out_sb = tailp.tile([1, Y], f32)
        nc.vector.tensor_copy(out_sb[:], outp[:])
        nc.scalar.dma_start(out[:], out_sb[:])

    nc.compile()
    return nc


def _shard_inputs(x, lam, a, c):
    """Per-core input maps. Slicing/layout/dtype transforms only."""
    m0_np = np.tile(np.eye(P, dtype=np.float32), (1, Y))          # [P, Y*P]
    ones_np = np.ones((P, 1), dtype=np.float32)
    in_maps = []
    for q in range(NCORES):
        lam_q = lam[:, q]                                          # [Y, P, D]
        lamT = np.ascontiguousarray(
            lam_q.transpose(2, 0, 1).reshape(DC, KC, YP)
            .transpose(1, 0, 2).reshape(KC, DC * YP)
        )
        x_q = x[q]                                                 # [P, D]
        xTn = np.ascontiguousarray(
            x_q.T.reshape(DC, KC, P).transpose(1, 0, 2).reshape(KC, DC * P)
        )
        aTn = np.ascontiguousarray(a[:, q].T)                      # [P, Y]
        crn = np.ascontiguousarray(
            np.broadcast_to(c[:, q][None, :], (P, Y))
        ).astype(np.float32)
        in_maps.append(
            {
                "lamT": lamT.astype(np.float16),
                "xT": xTn.astype(np.float16),
                "aT": aTn.astype(np.float32, copy=False),
                "crep": crn,
                "m0": m0_np,
                "ones": ones_np,
            }
        )
    return in_maps


def get_nc():
    key = (MODE, SLABS, BUFS, TAIL_PAIRS, XSPLIT)
    if key not in _CACHE:
        _CACHE[key] = _build_nc()
    return _CACHE[key]


def run(x, lam, a, c, trace=False, **spmd_kwargs):
    from concourse.bass_utils import run_bass_kernel_spmd

    nc = get_nc()
    in_maps = _shard_inputs(
        np.asarray(x), np.asarray(lam), np.asarray(a), np.asarray(c)
    )
    res = run_bass_kernel_spmd(
        nc, in_maps, core_ids=list(range(NCORES)), trace=trace, **spmd_kwargs
    )
    out = np.zeros((Y,), dtype=np.float32)
    for core_res in res.results:
        out += core_res["out"].reshape(Y)
    return out, res


def kernel(x, lam, a, c):
    out, _ = run(x, lam, a, c, trace=False)
    return out


# revision 27
# speedup vs baseline: 1.8069x; 1.0947x over previous
"""Trainium2 Bass kernel for the AdditiveModel reduction.

Computes out[y] = sum_{q,p} c[y,q] * a[y,q,p] * dot(lam[y,q,p,:], x[q,p,:])
with Y=16, Q=8, P=32, D=8192 (lam is 128 MiB -> memory-bound).

Sharding: one q per core (Q == 8 cores). Each core is fully independent and
produces a partial out[16]; the host sums the 8 partials at gather time.

Per-core compute: the D-axis dot products run on the TensorEngine. At
sharding time the host hands each core its lam slice pre-transposed to
[d, (y,p)] layout (d on partitions) and cast to fp16 (total output error
~1e-4 of output scale, comparable to the PE's fp32r path), so the dots
become 64 PSUM-accumulated matmuls lhsT=x[dchunk, p] (128x32),
rhs=lam[dchunk, (y,p)] (128x512) with fp32 accumulation. PSUM then holds
G[m, (y,p)] = dot(x[p_m,:], lam[y,p,:]); a masked diagonal extraction +
(c*a) weighting + ones-matvec collapse yields the 16 outputs.

Streaming: lam is split in chunk-halves across the two HWDGE rings (SP and
ACT) so both hardware descriptor generators run in parallel; the matmul
order interleaves the two streams. All slabs are SBUF-resident (8 MiB
fp16), so there is no slot-release gating. gpsimd SWDGE is avoided -- its
Q7 descriptor generation starts ~5us late.
"""

from contextlib import ExitStack

import numpy as np

Y, Q, P, D = 16, 8, 32, 8192
NCORES = 8
KC = 128                 # contraction chunk (partition count)
DC = D // KC             # 64 d-chunks
YP = Y * P               # 512
SLAB_CHUNKS = [4, 4, 4, 4, 4, 4, 4, 4]   # per-ring slab sizes in chunks

_CACHE = {}


def _build_nc():
    import concourse.bass as bass
    import concourse.mybir as mybir
    import concourse.tile as tile
    from concourse import bacc

    f32 = mybir.dt.float32
    f16 = mybir.dt.float16
    nc = bacc.Bacc(None, target_bir_lowering=False)

    lamT = nc.declare_dram_parameter("lamT", [KC, DC * YP], f16, isOutput=False)
    xT = nc.declare_dram_parameter("xT", [KC, DC * P], f16, isOutput=False)
    aT = nc.declare_dram_parameter("aT", [P, Y], f32, isOutput=False)
    crep = nc.declare_dram_parameter("crep", [P, Y], f32, isOutput=False)
    m0 = nc.declare_dram_parameter("m0", [P, YP], f32, isOutput=False)
    ones = nc.declare_dram_parameter("ones", [P, 1], f32, isOutput=False)
    out = nc.declare_dram_parameter("out", [1, Y], f32, isOutput=True)

    with tile.TileContext(nc) as tc, ExitStack() as ctx:
        const = ctx.enter_context(tc.tile_pool(name="const", bufs=1))
        slab_pool = ctx.enter_context(tc.tile_pool(name="slab", bufs=len(SLAB_CHUNKS)))
        psum_pool = ctx.enter_context(
            tc.tile_pool(name="psum", bufs=1, space=bass.MemorySpace.PSUM)
        )
        tailp = ctx.enter_context(tc.tile_pool(name="tail", bufs=1))

        # x loads first on the SP ring: it gates the first matmul.
        x_sb = const.tile([KC, DC * P], f16)
        nc.sync.dma_start(x_sb[:], xT[:])

        proj = psum_pool.tile([P, YP], f32)
        half = DC // 2
        assert sum(SLAB_CHUNKS) == half
        mm_seq = []
        lo = 0
        for s, cps in enumerate(SLAB_CHUNKS):
            slab_a = slab_pool.tile([KC, cps * YP], f16, tag="slab_a")
            a_lo = lo
            nc.sync.dma_start(slab_a[:], lamT[:, a_lo * YP:(a_lo + cps) * YP])
            slab_b = slab_pool.tile([KC, cps * YP], f16, tag="slab_b")
            b_lo = half + lo
            nc.scalar.dma_start(slab_b[:], lamT[:, b_lo * YP:(b_lo + cps) * YP])
            for c in range(cps):
                mm_seq.append((a_lo + c, slab_a[:, c * YP:(c + 1) * YP]))
                mm_seq.append((b_lo + c, slab_b[:, c * YP:(c + 1) * YP]))
            lo += cps

        for i, (cg, ap) in enumerate(mm_seq):
            nc.tensor.matmul(
                proj[:],
                x_sb[:, cg * P:(cg + 1) * P],
                ap,
                start=(i == 0),
                stop=(i == len(mm_seq) - 1),
            )

        # consts ride the ACT ring (needed only for the tail)
        m0_sb = const.tile([P, YP], f32)
        nc.scalar.dma_start(m0_sb[:], m0[:])
        aT_sb = const.tile([P, Y], f32)
        nc.scalar.dma_start(aT_sb[:], aT[:])
        cr_sb = const.tile([P, Y], f32)
        nc.scalar.dma_start(cr_sb[:], crep[:])
        on_sb = const.tile([P, 1], f32)
        nc.scalar.dma_start(on_sb[:], ones[:])
        wT = const.tile([P, Y], f32)
        nc.vector.tensor_mul(wT[:], aT_sb[:], cr_sb[:])

        # tail: diag mask keeps only m == p entries of G[m, (y,p)]
        t2 = tailp.tile([P, YP], f32)
        nc.vector.tensor_mul(t2[:], proj[:], m0_sb[:])
        # sum each 32-wide p-group -> S[m, y] = proj[y, m]
        s_t = tailp.tile([P, Y], f32)
        nc.vector.reduce_sum(
            s_t[:],
            t2[:].rearrange("m (y p) -> m y p", p=P),
            axis=mybir.AxisListType.X,
        )
        # weight by c*a and collapse partitions with a ones-matvec
        sw = tailp.tile([P, Y], f32)
        nc.vector.tensor_mul(sw[:], s_t[:], wT[:])
        outp = psum_pool.tile([1, Y], f32)
        nc.tensor.matmul(outp[:], on_sb[:], sw[:], start=True, stop=True)
        out_sb = tailp.tile([1, Y], f32)
        nc.vector.tensor_copy(out_sb[:], outp[:])
        nc.scalar.dma_start(out[:], out_sb[:])

    nc.compile()
    return nc


def _shard_inputs(x, lam, a, c):
    """Per-core input maps. Slicing/layout/dtype transforms only."""
    m0_np = np.tile(np.eye(P, dtype=np.float32), (1, Y))          # [P, Y*P]
    ones_np = np.ones((P, 1), dtype=np.float32)
    in_maps = []
    for q in range(NCORES):
        lam_q = lam[:, q]                                          # [Y, P, D]
        lamT = np.ascontiguousarray(
            lam_q.transpose(2, 0, 1).reshape(DC, KC, YP)
            .transpose(1, 0, 2).reshape(KC, DC * YP)
        )
        x_q = x[q]                                                 # [P, D]
        xTn = np.ascontiguousarray(
            x_q.T.reshape(DC, KC, P).transpose(1, 0, 2).reshape(KC, DC * P)
        )
        aTn = np.ascontiguousarray(a[:, q].T)                      # [P, Y]
        crn = np.ascontiguousarray(
            np.broadcast_to(c[:, q][None, :], (P, Y))
        ).astype(np.float32)
        in_maps.append(
            {
                "lamT": lamT.astype(np.float16),
                "xT": xTn.astype(np.float16),
                "aT": aTn.astype(np.float32, copy=False),
                "crep": crn,
                "m0": m0_np,
                "ones": ones_np,
            }
        )
    return in_maps


def get_nc():
    key = tuple(SLAB_CHUNKS)
    if key not in _CACHE:
        _CACHE[key] = _build_nc()
    return _CACHE[key]


def run(x, lam, a, c, trace=False, **spmd_kwargs):
    from concourse.bass_utils import run_bass_kernel_spmd

    nc = get_nc()
    in_maps = _shard_inputs(
        np.asarray(x, dtype=np.float32),
        np.asarray(lam, dtype=np.float32),
        np.asarray(a, dtype=np.float32),
        np.asarray(c, dtype=np.float32),
    )
    res = run_bass_kernel_spmd(
        nc, in_maps, core_ids=list(range(NCORES)), trace=trace, **spmd_kwargs
    )
    out = np.zeros((Y,), dtype=np.float32)
    for core_res in res.results:
        out += core_res["out"].reshape(Y)
    return out, res


def kernel(x, lam, a, c):
    try:
        out, _ = run(x, lam, a, c, trace=False)
    except Exception:
        # one retry to ride out transient device errors
        out, _ = run(x, lam, a, c, trace=False)
    return out


# revision 29
# speedup vs baseline: 1.8198x; 1.0072x over previous
"""Trainium2 Bass kernel for the AdditiveModel reduction.

Computes out[y] = sum_{q,p} c[y,q] * a[y,q,p] * dot(lam[y,q,p,:], x[q,p,:])
with Y=16, Q=8, P=32, D=8192 (lam is 128 MiB -> memory-bound).

Sharding: one q per core (Q == 8 cores). Each core is fully independent and
produces a partial out[16]; the host sums the 8 partials at gather time.

Per-core compute: the D-axis dot products run on the TensorEngine. At
sharding time the host hands each core its lam slice pre-transposed to
[d, (y,p)] layout (d on partitions) and cast to fp16 (total output error
~1e-4 of output scale, comparable to the PE's fp32r path), so the dots
become 64 PSUM-accumulated matmuls lhsT=x[dchunk, p] (128x32),
rhs=lam[dchunk, (y,p)] (128x512) with fp32 accumulation. PSUM then holds
G[m, (y,p)] = dot(x[p_m,:], lam[y,p,:]); a masked diagonal extraction +
(c*a) weighting + ones-matvec collapse yields the 16 outputs.

Streaming: lam is split in chunk-halves across the two HWDGE rings (SP and
ACT) so both hardware descriptor generators run in parallel; the matmul
order interleaves the two streams. All slabs are SBUF-resident (8 MiB
fp16), so there is no slot-release gating. gpsimd SWDGE is avoided -- its
Q7 descriptor generation starts ~5us late.
"""

from contextlib import ExitStack

import numpy as np

Y, Q, P, D = 16, 8, 32, 8192
NCORES = 8
KC = 128                 # contraction chunk (partition count)
DC = D // KC             # 64 d-chunks
YP = Y * P               # 512
SLAB_CHUNKS = [4, 4, 4, 4, 4, 4, 4, 4]   # per-ring slab sizes in chunks

_CACHE = {}


def _build_nc():
    import concourse.bass as bass
    import concourse.mybir as mybir
    import concourse.tile as tile
    from concourse import bacc

    f32 = mybir.dt.float32
    f16 = mybir.dt.float16
    nc = bacc.Bacc(None, target_bir_lowering=False)

    lamT = nc.declare_dram_parameter("lamT", [KC, DC * YP], f16, isOutput=False)
    xT = nc.declare_dram_parameter("xT", [KC, DC * P], f16, isOutput=False)
    aT = nc.declare_dram_parameter("aT", [P, Y], f32, isOutput=False)
    crep = nc.declare_dram_parameter("crep", [P, Y], f32, isOutput=False)
    m0 = nc.declare_dram_parameter("m0", [P, YP], f32, isOutput=False)
    ones = nc.declare_dram_parameter("ones", [P, 1], f32, isOutput=False)
    out = nc.declare_dram_parameter("out", [1, Y], f32, isOutput=True)

    with tile.TileContext(nc) as tc, ExitStack() as ctx:
        const = ctx.enter_context(tc.tile_pool(name="const", bufs=1))
        slab_pool = ctx.enter_context(tc.tile_pool(name="slab", bufs=len(SLAB_CHUNKS)))
        psum_pool = ctx.enter_context(
            tc.tile_pool(name="psum", bufs=1, space=bass.MemorySpace.PSUM)
        )
        tailp = ctx.enter_context(tc.tile_pool(name="tail", bufs=1))

        # x loads first on the SP ring: it gates the first matmul.
        x_sb = const.tile([KC, DC * P], f16)
        nc.sync.dma_start(x_sb[:], xT[:])

        proj = psum_pool.tile([P, YP], f32)
        half = DC // 2
        assert sum(SLAB_CHUNKS) == half
        mm_seq = []
        lo = 0
        for s, cps in enumerate(SLAB_CHUNKS):
            slab_a = slab_pool.tile([KC, cps * YP], f16, tag="slab_a")
            a_lo = lo
            nc.sync.dma_start(slab_a[:], lamT[:, a_lo * YP:(a_lo + cps) * YP])
            slab_b = slab_pool.tile([KC, cps * YP], f16, tag="slab_b")
            b_lo = half + lo
            nc.scalar.dma_start(slab_b[:], lamT[:, b_lo * YP:(b_lo + cps) * YP])
            for c in range(cps):
                mm_seq.append((a_lo + c, slab_a[:, c * YP:(c + 1) * YP]))
                mm_seq.append((b_lo + c, slab_b[:, c * YP:(c + 1) * YP]))
            lo += cps

        for i, (cg, ap) in enumerate(mm_seq):
            nc.tensor.matmul(
                proj[:],
                x_sb[:, cg * P:(cg + 1) * P],
                ap,
                start=(i == 0),
                stop=(i == len(mm_seq) - 1),
            )

        # consts ride the ACT ring (needed only for the tail)
        m0_sb = const.tile([P, YP], f32)
        nc.scalar.dma_start(m0_sb[:], m0[:])
        aT_sb = const.tile([P, Y], f32)
        nc.scalar.dma_start(aT_sb[:], aT[:])
        cr_sb = const.tile([P, Y], f32)
        nc.scalar.dma_start(cr_sb[:], crep[:])
        on_sb = const.tile([P, 1], f32)
        nc.scalar.dma_start(on_sb[:], ones[:])
        wT = const.tile([P, Y], f32)
        nc.vector.tensor_mul(wT[:], aT_sb[:], cr_sb[:])

        # tail: diag mask keeps only m == p entries of G[m, (y,p)]
        t2 = tailp.tile([P, YP], f32)
        nc.vector.tensor_mul(t2[:], proj[:], m0_sb[:])
        # sum each 32-wide p-group -> S[m, y] = proj[y, m]
        s_t = tailp.tile([P, Y], f32)
        nc.vector.reduce_sum(
            s_t[:],
            t2[:].rearrange("m (y p) -> m y p", p=P),
            axis=mybir.AxisListType.X,
        )
        # weight by c*a and collapse partitions with a ones-matvec
        sw = tailp.tile([P, Y], f32)
        nc.vector.tensor_mul(sw[:], s_t[:], wT[:])
        outp = psum_pool.tile([1, Y], f32)
        nc.tensor.matmul(outp[:], on_sb[:], sw[:], start=True, stop=True)
        out_sb = tailp.tile([1, Y], f32)
        nc.vector.tensor_copy(out_sb[:], outp[:])
        nc.scalar.dma_start(out[:], out_sb[:])

    nc.compile()
    return nc


def _shard_inputs(x, lam, a, c):
    """Per-core input maps. Slicing/layout/dtype transforms only."""
    m0_np = np.tile(np.eye(P, dtype=np.float32), (1, Y))          # [P, Y*P]
    ones_np = np.ones((P, 1), dtype=np.float32)
    in_maps = []
    for q in range(NCORES):
        lam_q = lam[:, q]                                          # [Y, P, D]
        lamT = np.ascontiguousarray(
            lam_q.transpose(2, 0, 1).reshape(DC, KC, YP)
            .transpose(1, 0, 2).reshape(KC, DC * YP)
        )
        x_q = x[q]                                                 # [P, D]
        xTn = np.ascontiguousarray(
            x_q.T.reshape(DC, KC, P).transpose(1, 0, 2).reshape(KC, DC * P)
        )
        aTn = np.ascontiguousarray(a[:, q].T)                      # [P, Y]
        crn = np.ascontiguousarray(
            np.broadcast_to(c[:, q][None, :], (P, Y))
        ).astype(np.float32)
        in_maps.append(
            {
                "lamT": lamT.astype(np.float16),
                "xT": xTn.astype(np.float16),
                "aT": aTn.astype(np.float32, copy=False),
                "crep": crn,
                "m0": m0_np,
                "ones": ones_np,
            }
        )
    return in_maps


def get_nc():
    key = tuple(SLAB_CHUNKS)
    if key not in _CACHE:
        _CACHE[key] = _build_nc()
    return _CACHE[key]


def run(x, lam, a, c, trace=False, **spmd_kwargs):
    from concourse.bass_utils import run_bass_kernel_spmd

    nc = get_nc()
    in_maps = _shard_inputs(
        np.asarray(x, dtype=np.float32),
        np.asarray(lam, dtype=np.float32),
        np.asarray(a, dtype=np.float32),
        np.asarray(c, dtype=np.float32),
    )
    res = run_bass_kernel_spmd(
        nc, in_maps, core_ids=list(range(NCORES)), trace=trace, **spmd_kwargs
    )
    out = np.zeros((Y,), dtype=np.float32)
    for core_res in res.results:
        out += core_res["out"].reshape(Y)
    return out, res


def kernel(x, lam, a, c):
    try:
        out, _ = run(x, lam, a, c, trace=False)
    except Exception:
        # one retry to ride out transient device errors
        out, _ = run(x, lam, a, c, trace=False)
    return out
